# revision 2
# baseline (speedup 1.0000x reference)
"""Trainium2 Bass kernel for ChunkedGeoSparseLinear (gather-mode sparse linear).

out[n, o] = sum_k x[n, idx[o, k]] * w[o, k] + b[o]
  x: (4096, 4096) f32, idx: (4096, 16) i64, w: (4096, 16) f32, b: (4096,) f32

Strategy (data-parallel over OUTPUTS, 8 cores):
  - Host: transpose x -> xT [feat, row], cast to fp8 e3m4 (halves gather
    bytes; rel-err ~1.5e-2 < 2e-2 tol), replicate to all cores.
  - Core d owns outputs [512d, 512d+512): 8192 taps, each tap = one full
    4096-row feature line (4 KiB descriptor -> HBM line rate; 8 dma_gather
    calls of 1024 taps each instead of 128 small ones -> SWDGE emission
    cost drops ~12x).
  - PE: per 128-output group, 16 diagonal bf16 lhsT matmuls (tap m of all
    128 outs) x fp8 rhs accumulate into PSUM [128, 512] column chunks.
  - ScalarE drains PSUM with bias add to fp16; host transposes/casts back.
"""

import sys

import numpy as np
import ml_dtypes

for _p in ("/opt/trn_rl_repo", "/opt/pypackages"):
    if _p not in sys.path:
        sys.path.append(_p)

N = 4096
IN_F = 4096
OUT_F = 4096
K = 16
NCORES = 8
OSLAB = OUT_F // NCORES       # 512 outputs per core
NGRP = OSLAB // 128           # 4 psum groups of 128 outputs
TAPS = OSLAB * K              # 8192 taps per core
TPC = 1024                    # taps per dma_gather call (SWDGE ring limit)
CPG = (128 * K) // TPC        # gather calls per psum group (2)
NCH = N // 512                # 8 psum column chunks of 512 rows

_CACHE = {}


def _build(reps: int = 1):
    """Build + compile the per-core Bass program (SPMD: same program, 8 cores)."""
    import concourse.bacc as bacc
    import concourse.mybir as mybir
    import concourse.tile as tile

    dt = mybir.dt
    nc = bacc.Bacc("TRN2", debug=False, num_devices=NCORES,
                   enable_partition_id=False, num_swdge_queues=4)

    xt = nc.dram_tensor("xt", [IN_F, N], dt.float8e3, kind="ExternalInput")
    idxs = nc.dram_tensor("idxs", [128, TAPS // 16], dt.int16, kind="ExternalInput")
    wcol = nc.dram_tensor("wcol", [128, NGRP * K], dt.bfloat16, kind="ExternalInput")
    bias = nc.dram_tensor("bias", [128, NGRP], dt.float32, kind="ExternalInput")
    ident_d = nc.dram_tensor("ident", [128, 128], dt.bfloat16, kind="ExternalInput")
    outT = nc.dram_tensor("outT", [OSLAB, N], dt.float16, kind="ExternalOutput")
    # reps-dependent output shape keeps timing variants from aliasing in the
    # executable cache (the cache key ignores the embedded BIR)
    nc.dram_tensor("repstag", [1, reps], dt.float32, kind="ExternalOutput")

    with tile.TileContext(nc) as tc:
        with (
            tc.tile_pool(name="singles", bufs=1) as singles,
            tc.tile_pool(name="gpool", bufs=4) as gpool,
            tc.tile_pool(name="dpool", bufs=2) as dpool,
            tc.tile_pool(name="ppool", bufs=6, space="PSUM") as ppool,
            tc.tile_pool(name="opool", bufs=2) as opool,
        ):
            idxs_sb = singles.tile([128, TAPS // 16], dt.int16)
            nc.sync.dma_start(idxs_sb[:], idxs[:])
            w_sb = singles.tile([128, NGRP * K], dt.bfloat16)
            nc.sync.dma_start(w_sb[:], wcol[:])
            bias_sb = singles.tile([128, NGRP], dt.float32)
            nc.sync.dma_start(bias_sb[:], bias[:])
            ident = singles.tile([128, 128], dt.bfloat16)
            nc.sync.dma_start(ident[:], ident_d[:])

            def body(_i=None):
                ident_b = ident[:].unsqueeze(1).broadcast_to([128, K, 128])
                tiles_per_call = TPC // 128      # 8 tap tiles per gather call
                for g in range(NGRP):
                    gs = []
                    for c in range(CPG):
                        call = g * CPG + c
                        gt = gpool.tile([128, tiles_per_call, N], dt.float8e3)
                        nc.gpsimd.dma_gather(
                            gt[:], xt[:],
                            idxs_sb[:, call * (TPC // 16):(call + 1) * (TPC // 16)],
                            TPC, TPC, N,
                            queue_num=call % 4,
                        )
                        gs.append(gt)
                    diag = dpool.tile([128, K, 128], dt.bfloat16)
                    wb = (w_sb[:, g * K:(g + 1) * K]
                          .unsqueeze(2).broadcast_to([128, K, 128]))
                    nc.vector.tensor_tensor(diag[:], ident_b, wb,
                                            op=mybir.AluOpType.mult)
                    o = opool.tile([128, N], dt.float16)
                    for ch in range(NCH):
                        p = ppool.tile([128, 512], dt.float32)
                        for m in range(K):
                            nc.tensor.matmul(
                                p[:], diag[:, m, :],
                                gs[m // tiles_per_call][
                                    :, m % tiles_per_call,
                                    ch * 512:(ch + 1) * 512],
                                start=(m == 0), stop=(m == K - 1))
                        nc.scalar.activation(
                            o[:, ch * 512:(ch + 1) * 512], p[:],
                            mybir.ActivationFunctionType.Identity,
                            bias=bias_sb[:, g:g + 1])
                    nc.sync.dma_start(outT[g * 128:(g + 1) * 128, :], o[:])

            if reps == 1:
                body()
            else:
                with tc.For_i(0, reps, 1):
                    body()

    nc.compile()
    return nc


def _prep_inputs(x, in_index_per_out, weight, bias):
    """Host-side data prep: replicated fp8 xT + per-core tap tables."""
    idx = np.asarray(in_index_per_out).astype(np.int64)
    w = np.asarray(weight).astype(np.float32)
    b = np.asarray(bias).astype(np.float32)

    xT = np.ascontiguousarray(np.asarray(x).astype(np.float32).T
                              .astype(ml_dtypes.float8_e3m4))  # (IN_F, N)

    idxs_l, wcol_l, bias_l = [], [], []
    for d in range(NCORES):
        sl = slice(d * OSLAB, (d + 1) * OSLAB)
        idx_d = idx[sl]                        # (512, 16)
        w_d = w[sl]                            # (512, 16)
        b_d = b[sl]                            # (512,)
        # tap order: call (g, c) covers slots m = 8c..8c+7 of outputs
        # 128g..128g+127; within a call, tile j / partition p = tap
        # (out 128g+p, slot 8c+j), flat position j*128 + p.
        flat = (idx_d.reshape(NGRP, 128, K)      # [g, p, m]
                .transpose(0, 2, 1)              # [g, m, p]
                .reshape(NGRP, CPG, K // CPG, 128)  # [g, c, j, p]
                .reshape(-1))
        wrap = flat.reshape(-1, 16).T            # [16, TAPS//16]
        idxs_l.append(np.tile(wrap, (8, 1)).astype(np.int16))
        wcol_l.append(np.ascontiguousarray(
            w_d.reshape(NGRP, 128, K).transpose(1, 0, 2)
            .reshape(128, NGRP * K)).astype(ml_dtypes.bfloat16))
        bias_l.append(np.ascontiguousarray(
            b_d.reshape(NGRP, 128).T))           # [128, NGRP]
    ident_np = np.eye(128, dtype=ml_dtypes.bfloat16)
    return xT, idxs_l, wcol_l, bias_l, ident_np


def kernel(x, in_index_per_out, weight, bias):
    from concourse import bass_utils

    xT, idxs_l, wcol_l, bias_l, ident_np = _prep_inputs(
        x, in_index_per_out, weight, bias)

    if "nc" not in _CACHE:
        _CACHE["nc"] = _build(reps=1)
    nc = _CACHE["nc"]

    in_maps = [
        {"xt": xT, "idxs": idxs_l[d], "wcol": wcol_l[d],
         "bias": bias_l[d], "ident": ident_np}
        for d in range(NCORES)
    ]
    res = bass_utils.run_bass_kernel_spmd(nc, in_maps,
                                          core_ids=list(range(NCORES)))
    out = np.empty((N, OUT_F), dtype=np.float32)
    for d in range(NCORES):
        out[:, d * OSLAB:(d + 1) * OSLAB] = \
            res.results[d]["outT"].astype(np.float32).T
    return out
